# revision 40
# baseline (speedup 1.0000x reference)
"""Trainium2 Bass kernel for nn_Attention (B=2, C=256, H=W=64, 8 heads).

Sharding: 8 cores = 2 batches x 4 query-chunks (1024 queries each), no
collectives. Each core gets its batch's full x (bf16) with token columns
rolled so its own query chunk sits at columns 0:1024 (attention is
permutation-invariant over keys); it computes LN + projections + attention
for its queries and writes a [256, 1024] slice of the output.

v3 structure (DoubleRow fp8 S matmuls; ~390us -> ~2xx us sim):
- x is pre-scaled by rstd (x_rstd = x * rstd broadcast) so every projection
  PSUM holds the final value directly: W @ x_rstd - rowsum(W) x (mu*rstd)
  + (W@beta) x 1. Projection evacuations become plain dtype-converting
  copies (no per-token multiply), freeing DVE time.
- K and Q projections are emitted in a folded [64, 2, 512] PSUM layout
  (channel ch = p + 64*i) by splitting each projection into two 64-column
  stationaries. Evacuated to fp8 [128, 2, N] tiles (upper 64 partitions
  zero), which feed full-array *unmasked* DoubleRow fp8 S matmuls:
  256 cycles per [128 keys, 512 queries] head-tile — 2x the bf16 rate,
  while still counting as PE activity for the HAM clock gate (masked
  tile_position matmuls don't, and would drop the whole phase to 1.2 GHz).
- Wq and Wk each carry sqrt(128*log2e*attn_scale) so PSUM logits keep the
  Schraudolph convention: pair-0 exp via f32->i16 convert bit trick on
  VectorE, pair-1 true Exp on ScalarE, both [128,1024] per key-chunk.
- S pair-0 PSUM is double-buffered (pair-1 single) so the j-loop period is
  set by engine capacity (~1.3us: PE 4xDR-S + 4xAV, DVE exp, ACT exp), not
  the S->exp->S bank turnaround.
- AV keeps the 33rd dummy-V-channel trick (bias row pairs the ones row) so
  softmax denominators accumulate for free; AV for chunk j is emitted after
  the S matmuls of chunk j+1 (software pipelining) so the exp engines stay
  off the PE's serial path. Accumulators are evacuated RAW (valid partition
  ranges only) to per-(hg,pr) attnT tiles whose garbage rows are zeroed
  once; normalization (bit-trick reciprocal of the denominator rows, DMA
  hop to partition 0, gpsimd partition_broadcast, multiply) happens off the
  critical path. NOTE: hw partition_broadcast reads the tile's partition 0
  and ignores the AP base partition — the DMA hop is required (reading the
  row at partition 32/96 directly returns uninitialized memory on hw, sim
  does not model this). The output projection contracts the four attnT
  tiles with zero-padded reordered W_proj tiles at the very end.

v3 measured: 248805 ns CoreSim (baseline 390555 sim / 443752 ns HW);
rel err 1.65e-4 on hardware, stable across runs. K/Q projections contract
all 256 input channels in ONE DoubleRow matmul each (x*rstd is written as
a ci-folded [128, 2, N] fp8 tile, weights ship host-folded fp8), so a
projection group is one 107 ns DR matmul plus the rank-2 fixup.
"""

import numpy as np

B, C, H, W = 2, 256, 64, 64
N = H * W            # 4096 tokens
NH, HD = 8, 32       # heads, head_dim
NQ = N // 4          # queries per core
LN_EPS = 1e-5
LOG2E = 1.4426950408889634
LN2 = 0.6931471805599453
ATTN_SCALE = HD ** -0.5
A_SCALE = 128.0 * LOG2E * ATTN_SCALE   # total logit scale in PSUM
SQA = A_SCALE ** 0.5                   # folded into each of Wq, Wk
B16F = 16256.0 - 5.6                   # Schraudolph bias (calibrated)
KRSQ = 24375.25                        # bf16 bit-trick rsqrt bias (<=3.7% rel)
KRCP = 32498.75                        # bf16 bit-trick recip bias (<=5.3% rel)

_PROFILE = False
_DEBUG = False
_CACHE = {}


def _build():
    from concourse import bacc
    from concourse import mybir
    import concourse.tile as tile

    f32 = mybir.dt.float32
    bf16 = mybir.dt.bfloat16
    fp8 = mybir.dt.float8e4
    i16 = mybir.dt.int16
    ALU = mybir.AluOpType
    ACTF = mybir.ActivationFunctionType
    DR = mybir.MatmulPerfMode.DoubleRow

    nc = bacc.Bacc("TRN2", target_bir_lowering=False)
    xbd = nc.dram_tensor("xb", [C, N], bf16, kind="ExternalInput")
    xfd = nc.dram_tensor("xf", [C, NQ], f32, kind="ExternalInput")
    wqd = nc.dram_tensor("wq8", [128, 2 * C], fp8, kind="ExternalInput")  # gamma+SQA, ci-folded
    wkd = nc.dram_tensor("wk8", [128, 2 * C], fp8, kind="ExternalInput")
    wvd = nc.dram_tensor("wvT", [C, NH * 33], fp8, kind="ExternalInput")
    wpxd = nc.dram_tensor("wpx", [4 * 128, C], bf16, kind="ExternalInput")
    # rank-1 LN fixup pairs: row0 = W@beta (pairs ones), row1 = -rowsum(W')
    # (pairs mu*rstd) — one contract-2 fixup matmul per projection chunk
    wbqd = nc.dram_tensor("wbq", [2, C], bf16, kind="ExternalInput")
    wbkd = nc.dram_tensor("wbk", [2, C], bf16, kind="ExternalInput")
    wbvd = nc.dram_tensor("wbv", [2, NH * 33], bf16, kind="ExternalInput")
    bpd = nc.dram_tensor("bp", [C, 1], f32, kind="ExternalInput")
    od = nc.dram_tensor("out", [C, NQ], f32, kind="ExternalOutput")
    if _DEBUG:
        dbg = {
            "d_kT0": nc.dram_tensor("d_kT0", [128, 2 * N], fp8, kind="ExternalOutput"),
            "d_qp0": nc.dram_tensor("d_qp0", [128, 2 * NQ], fp8, kind="ExternalOutput"),
            "d_musrt2": nc.dram_tensor("d_musrt2", [2, N], bf16, kind="ExternalOutput"),
            "d_vsb": nc.dram_tensor("d_vsb", [128, 32 * NH * 33], bf16, kind="ExternalOutput"),
            "d_attx0": nc.dram_tensor("d_attx0", [128, NQ], bf16, kind="ExternalOutput"),
            "d_attx1": nc.dram_tensor("d_attx1", [128, NQ], bf16, kind="ExternalOutput"),
            "d_attx2": nc.dram_tensor("d_attx2", [128, NQ], bf16, kind="ExternalOutput"),
            "d_attx3": nc.dram_tensor("d_attx3", [128, NQ], bf16, kind="ExternalOutput"),
            "d_rsbf": nc.dram_tensor("d_rsbf", [1, N], bf16, kind="ExternalOutput"),
            "d_kT1": nc.dram_tensor("d_kT1", [128, 2 * N], fp8, kind="ExternalOutput"),
            "d_qp4": nc.dram_tensor("d_qp4", [128, 2 * NQ], fp8, kind="ExternalOutput"),
            "d_qp7": nc.dram_tensor("d_qp7", [128, 2 * NQ], fp8, kind="ExternalOutput"),
        }

    with tile.TileContext(nc) as tc:
        with tc.tile_pool(name="big", bufs=1) as big, \
             tc.tile_pool(name="sml", bufs=4) as sml:

            # ---- load inputs ----
            xb = [big.tile([128, N], bf16, tag=f"xb{c}", name=f"xb{c}") for c in range(2)]
            for q4 in range(4):
                qs = slice(q4 * 1024, (q4 + 1) * 1024)
                for c in range(2):
                    nc.sync.dma_start(out=xb[c][:, qs], in_=xbd[c * 128:(c + 1) * 128, qs])
            xf = [big.tile([128, NQ], f32, tag=f"xf{c}", name=f"xf{c}") for c in range(2)]
            for c in range(2):
                nc.sync.dma_start(out=xf[c][:, :], in_=xfd[c * 128:(c + 1) * 128, :])
            w_sb = {}
            for name, t in (("q", wqd), ("k", wkd)):
                s = big.tile([128, 2, C], fp8, tag=f"w{name}8", name=f"w{name}8")
                nc.sync.dma_start(out=s[:, :, :].rearrange("p a b -> p (a b)"), in_=t[:, :])
                w_sb[name] = s
            for ci in range(2):
                s = big.tile([128, NH * 33], fp8, tag=f"wv{ci}", name=f"wv{ci}")
                nc.sync.dma_start(out=s[:, :], in_=wvd[ci * 128:(ci + 1) * 128, :])
                w_sb["v", ci] = s
            wpx_sb = [big.tile([128, C], bf16, tag=f"wpx{t}", name=f"wpx{t}") for t in range(4)]
            for t in range(4):
                nc.sync.dma_start(out=wpx_sb[t][:, :], in_=wpxd[t * 128:(t + 1) * 128, :])
            wb_sb = {}
            for name, t, nout in (("q", wbqd, C), ("k", wbkd, C), ("v", wbvd, NH * 33)):
                s = big.tile([2, nout], bf16, tag=f"wb{name}", name=f"wb{name}")
                nc.sync.dma_start(out=s[:, :], in_=t[:, :])
                wb_sb[name] = s
            bp_sb = [big.tile([128, 1], f32, tag=f"bp{c}", name=f"bp{c}") for c in range(2)]
            for c in range(2):
                nc.sync.dma_start(out=bp_sb[c][:, :], in_=bpd[c * 128:(c + 1) * 128, :])

            onesC = big.tile([128, 1], bf16, tag="onesC", name="onesC")
            nc.vector.memset(onesC[:, :], 1.0 / C)
            ones_row = big.tile([1, 128], bf16, tag="onesr", name="onesr")
            nc.vector.memset(ones_row[:, :], 1.0)

            mu_row = big.tile([1, N], bf16, tag="murow", name="murow")
            rs_bf = big.tile([1, N], bf16, tag="rsbf", name="rsbf")
            murs_row = big.tile([1, N], bf16, tag="mursrow", name="mursrow")
            musrt2 = big.tile([2, N], bf16, tag="musrt2", name="musrt2")
            nc.vector.memset(musrt2[0:1, :], 1.0)
            rs_ball = big.tile([128, N], bf16, tag="rsball", name="rsball")
            xr2 = big.tile([128, 2, N], fp8, tag="xr2", name="xr2")
            xsq = [big.tile([128, N], bf16, tag=f"xsq{c}", name=f"xsq{c}") for c in range(2)]

            # folded fp8 K (ch = p + 64*i within hg block; upper 64 partitions 0)
            kT = [big.tile([128, 2, N], fp8, tag=f"kT{c}", name=f"kT{c}") for c in range(2)]
            for c in range(2):
                nc.gpsimd.memset(kT[c][:, :, :], 0.0)
            # folded fp8 padded Q per head (only 32 rows of one i-half nonzero)
            qp = [big.tile([128, 2, NQ], fp8, tag=f"qp{h}", name=f"qp{h}") for h in range(NH)]
            for h in range(NH):
                nc.gpsimd.memset(qp[h][:, :, :], 0.0)
            v_sb = big.tile([128, 32, NH, 33], bf16, tag="v", name="v")
            # attn output tiles, rows 0:33 & 64:97 valid (head pair + denoms)
            attx = [big.tile([128, NQ], bf16, tag=f"at{t}", name=f"at{t}") for t in range(4)]
            for t in range(4):
                nc.gpsimd.memset(attx[t][:, :], 0.0)
            rcpT = big.tile([128, NQ], bf16, tag="rcpT", name="rcpT")

            # ---- LN stats + projections ----
            with tc.tile_pool(name="lnsb", bufs=1) as lnsb, \
                 tc.tile_pool(name="lnp", bufs=2, space="PSUM") as lnp, \
                 tc.tile_pool(name="rsp", bufs=1, space="PSUM") as rsp, \
                 tc.tile_pool(name="mmk", bufs=3, space="PSUM") as mmk, \
                 tc.tile_pool(name="mm", bufs=2, space="PSUM") as mmp:
                # full-array warmup matmuls on early-arriving weight tiles:
                # releases the HAM clock-gate to 2.4 GHz during the x DMA wait
                for w in range(20):
                    wps = mmp.tile([128, NH * 33], f32, tag="vproj", name="warm")
                    nc.tensor.matmul(wps[:, :], wpx_sb[w % 4][:, 0:128],
                                     w_sb["v", w % 2][:, :], start=True, stop=True)
                for hh in range(2):
                    hl = slice(hh * 2048, (hh + 1) * 2048)
                    for c in range(2):
                        nc.vector.tensor_tensor(xsq[c][:, hl], xb[c][:, hl],
                                                xb[c][:, hl], ALU.mult)
                # stats matmuls for all chunks; evacuate mu/sumsq rows to SBUF
                ssum = lnsb.tile([1, N], bf16, tag="ssum", name="ssum")
                for f in range(8):
                    fl = slice(f * 512, (f + 1) * 512)
                    mps = lnp.tile([1, 512], f32, tag="st", name="mps")
                    nc.tensor.matmul(mps[:, :], onesC[:, :], xb[0][:, fl], start=True, stop=False)
                    nc.tensor.matmul(mps[:, :], onesC[:, :], xb[1][:, fl], start=False, stop=True)
                    sps = lnp.tile([1, 512], f32, tag="st", name="sps")
                    nc.tensor.matmul(sps[:, :], onesC[:, :], xsq[0][:, fl], start=True, stop=False)
                    nc.tensor.matmul(sps[:, :], onesC[:, :], xsq[1][:, fl], start=False, stop=True)
                    nc.vector.tensor_copy(mu_row[0:1, fl], mps[:, :])
                    nc.scalar.copy(ssum[0:1, fl], sps[:, :])
                # one-shot whole-row LN math, in halves so the first half's
                # projections can start while the second half still chains
                mu2 = lnsb.tile([1, N], bf16, tag="mu2", name="mu2")
                vare = lnsb.tile([1, N], bf16, tag="vare", name="vare")
                for hh in range(2):
                    hl = slice(hh * 2048, (hh + 1) * 2048)
                    nc.vector.tensor_tensor(mu2[0:1, hl], mu_row[0:1, hl],
                                            mu_row[0:1, hl], ALU.mult)
                    nc.vector.tensor_tensor(vare[0:1, hl], ssum[0:1, hl],
                                            mu2[0:1, hl], ALU.subtract)
                    # rstd via bf16 exponent bit trick on VectorE
                    nc.vector.tensor_scalar(rs_bf[0:1, hl].bitcast(i16),
                                            vare[0:1, hl].bitcast(i16),
                                            -0.5, KRSQ, ALU.mult, ALU.add)
                    # mu*rstd -> musrt2 row1 (engines cannot write partition 1,
                    # but an SBUF->SBUF DMA can)
                    nc.vector.tensor_tensor(murs_row[0:1, hl], mu_row[0:1, hl],
                                            rs_bf[0:1, hl], ALU.mult)
                    nc.sync.dma_start(out=musrt2[1:2, hl], in_=murs_row[0:1, hl])
                # rstd broadcast + x*rstd for every chunk, ahead of projections
                for f in range(8):
                    fl = slice(f * 512, (f + 1) * 512)
                    rsb_ps = rsp.tile([128, 512], f32, tag="rsb", name="rsb")
                    nc.tensor.matmul(rsb_ps[:, :], ones_row[:, :], rs_bf[0:1, fl],
                                     start=True, stop=True)
                    nc.scalar.copy(rs_ball[:, fl], rsb_ps[:, :])
                    for c in range(2):
                        nc.vector.tensor_tensor(xr2[:, c, fl], xb[c][:, fl],
                                                rs_ball[:, fl], ALU.mult)
                for f in range(8):
                    fl = slice(f * 512, (f + 1) * 512)
                    # K projection, folded [64, 512] per (hg, i) (ch = p + 64*i)
                    for hg in range(2):
                        for i in range(2):
                            ps = mmk.tile([64, 512], f32, tag="proj", name="kproj")
                            cs = slice(hg * 128 + i * 64, hg * 128 + (i + 1) * 64)
                            nc.tensor.matmul(ps[:, :], w_sb["k"][:, :, cs],
                                             xr2[:, :, fl], perf_mode=DR,
                                             start=True, stop=False)
                            nc.tensor.matmul(ps[:, :], wb_sb["k"][:, cs],
                                             musrt2[:, fl], start=False, stop=True)
                            nc.scalar.copy(kT[hg][0:64, i, fl], ps[:, :])

                    # Q projection (first two chunks = this core's queries)
                    if f < 2:
                        for hg in range(2):
                            for i in range(2):
                                ps = mmk.tile([64, 512], f32, tag="proj", name="qproj")
                                cs = slice(hg * 128 + i * 64, hg * 128 + (i + 1) * 64)
                                nc.tensor.matmul(ps[:, :], w_sb["q"][:, :, cs],
                                                 xr2[:, :, fl], perf_mode=DR,
                                                 start=True, stop=False)
                                nc.tensor.matmul(ps[:, :], wb_sb["q"][:, cs],
                                                 musrt2[:, fl], start=False, stop=True)
                                # heads 2i, 2i+1 of hg live in rows 0:32, 32:64
                                for mh in range(2):
                                    m = i * 2 + mh
                                    rr = slice(32 * mh, 32 * mh + 32)
                                    nc.vector.tensor_copy(qp[hg * 4 + m][rr, i, fl],
                                                          ps[rr, :])

                # V projection per 128-token chunk (tokens in partitions). The
                # 33rd "dummy" channel per head has zero weights and rank-1
                # bias = 1 (pairs the ones row), so it accumulates the softmax
                # denominator during AV.
                for j in range(32):
                    jl = slice(j * 128, (j + 1) * 128)
                    ps = mmp.tile([128, NH * 33], f32, tag="vproj", name="vproj")
                    for ci in range(2):
                        nc.tensor.matmul(ps[:, :], xr2[:, ci, jl], w_sb["v", ci][:, :],
                                         start=(ci == 0), stop=False)
                    nc.tensor.matmul(ps[:, :], musrt2[:, jl],
                                     wb_sb["v"][:, :], start=False, stop=True)
                    nc.scalar.copy(v_sb[:, j, :, :],
                                   ps[:, :].rearrange("p (h e) -> p h e", h=NH))

            # ---- attention ----
            with tc.tile_pool(name="ssp", bufs=3, space="PSUM") as ssp, \
                 tc.tile_pool(name="avp", bufs=1, space="PSUM") as avp, \
                 tc.tile_pool(name="pp", bufs=3) as ppool, \
                 tc.tile_pool(name="nrm", bufs=4) as nrm:
                for f in range(2):
                    fl = slice(f * 512, (f + 1) * 512)
                    for hg in range(2):
                        av = [avp.tile([128, 512], f32, tag=f"av{pr}", name=f"av{pr}")
                              for pr in range(2)]

                        def emit_av(j, pt):
                            for pr in range(2):
                                for t2 in range(2):
                                    h = pr * 2 + t2
                                    nc.tensor.matmul(
                                        av[pr][t2 * 64:t2 * 64 + 33, :],
                                        v_sb[:, j, hg * 4 + h, :],
                                        pt[pr][:, t2 * 512:(t2 + 1) * 512],
                                        start=(j == 0), stop=(j == 31),
                                        skip_group_check=True,
                                        tile_position=(0, t2 * 64))

                        # software-pipelined: AV for chunk j is emitted after
                        # the S matmuls of chunk j+1 so the exp engines are
                        # never on the PE's serial path
                        pending = None
                        for j in range(32):
                            jl = slice(j * 128, (j + 1) * 128)
                            ss = [ssp.tile([128, 1024], f32, tag="ss", name="s0"),
                                  ssp.tile([128, 1024], f32, tag="ss", name="s1")]
                            pt = [ppool.tile([128, 1024], bf16, tag=f"p{i}", name=f"p{i}")
                                  for i in range(2)]
                            for pr in range(2):
                                for t2 in range(2):
                                    h = hg * 4 + pr * 2 + t2
                                    nc.tensor.matmul(ss[pr][:, t2 * 512:(t2 + 1) * 512],
                                                     kT[hg][:, :, jl], qp[h][:, :, fl],
                                                     perf_mode=DR, start=True, stop=True)
                            # pair 0: Schraudolph on VectorE; pair 1: ScalarE Exp
                            nc.vector.tensor_scalar(pt[0][:, :].bitcast(i16), ss[0][:, :],
                                                    B16F, None, ALU.add)
                            nc.scalar.activation(pt[1][:, :], ss[1][:, :],
                                                 ACTF.Exp, scale=LN2 / 128.0)
                            if pending is not None:
                                emit_av(*pending)
                            pending = (j, pt)
                        emit_av(*pending)
                        # raw evacuation (valid rows only; garbage rows stay 0)
                        for pr in range(2):
                            t = hg * 2 + pr
                            nc.scalar.copy(attx[t][0:33, fl], av[pr][0:33, :])
                            nc.scalar.copy(attx[t][64:97, fl], av[pr][64:97, :])
                            # denominator reciprocals (bf16 bit trick), then
                            # 0-stride-DMA broadcast + multiply off critical path
                            for t2 in range(2):
                                r = 32 + 64 * t2
                                nc.vector.tensor_scalar(rcpT[r:r + 1, fl].bitcast(i16),
                                                        attx[t][r:r + 1, fl].bitcast(i16),
                                                        -1.0, KRCP, ALU.mult, ALU.add)
                                # hop the reciprocal row to partition 0 via DMA
                                # (hw partition_broadcast reads the tile's
                                # partition 0, ignoring the AP base partition)
                                rcp0 = nrm.tile([1, 512], bf16, tag="rcp0", name="rcp0")
                                nc.sync.dma_start(out=rcp0[:, :], in_=rcpT[r:r + 1, fl])
                                bcs = nrm.tile([128, 512], bf16, tag="bcs", name="bcs")
                                nc.gpsimd.partition_broadcast(bcs[:, :], rcp0[:, :])
                                nc.vector.tensor_tensor(attx[t][64 * t2:64 * t2 + 32, fl],
                                                        attx[t][64 * t2:64 * t2 + 32, fl],
                                                        bcs[64 * t2:64 * t2 + 32, :], ALU.mult)

            if _DEBUG:
                nc.sync.dma_start(out=dbg["d_kT0"][:, :],
                                  in_=kT[0][:, :, :].rearrange("p a b -> p (a b)"))
                nc.sync.dma_start(out=dbg["d_qp0"][:, :],
                                  in_=qp[0][:, :, :].rearrange("p a b -> p (a b)"))
                nc.sync.dma_start(out=dbg["d_musrt2"][:, :], in_=musrt2[:, :])
                nc.sync.dma_start(out=dbg["d_vsb"][:, :],
                                  in_=v_sb[:, :, :, :].rearrange("p a b c -> p (a b c)"))
                for _t in range(4):
                    nc.sync.dma_start(out=dbg[f"d_attx{_t}"][:, :], in_=attx[_t][:, :])
                nc.sync.dma_start(out=dbg["d_rsbf"][:, :], in_=rs_bf[:, :])
                nc.sync.dma_start(out=dbg["d_kT1"][:, :],
                                  in_=kT[1][:, :, :].rearrange("p a b -> p (a b)"))
                nc.sync.dma_start(out=dbg["d_qp4"][:, :],
                                  in_=qp[4][:, :, :].rearrange("p a b -> p (a b)"))
                nc.sync.dma_start(out=dbg["d_qp7"][:, :],
                                  in_=qp[7][:, :, :].rearrange("p a b -> p (a b)"))

            # ---- output projection + bias + residual ----
            with tc.tile_pool(name="mm2", bufs=2, space="PSUM") as mm2, \
                 tc.tile_pool(name="ot", bufs=4) as otp:
                for mo in range(2):
                    ms = slice(mo * 128, (mo + 1) * 128)
                    for fh in range(2):
                        fl = slice(fh * 512, (fh + 1) * 512)
                        ps = mm2.tile([128, 512], f32, tag="o", name="o")
                        for t in range(4):
                            nc.tensor.matmul(ps[:, :], wpx_sb[t][:, ms],
                                             attx[t][:, fl], start=(t == 0), stop=(t == 3))
                        ot = otp.tile([128, 512], f32, tag="ot", name="ot")
                        nc.vector.scalar_tensor_tensor(ot[:, :], ps[:, :], bp_sb[mo][:, :],
                                                       xf[mo][:, fl], ALU.add, ALU.add)
                        nc.sync.dma_start(out=od[ms, fl], in_=ot[:, :])

    nc.finalize()
    return nc


def _prep_in_maps(x, ln_gamma, ln_beta, w_qkv, w_proj, b_proj):
    import ml_dtypes

    bf = ml_dtypes.bfloat16
    x = np.asarray(x, np.float32)
    w_qkv = np.asarray(w_qkv, np.float32)
    gam = np.asarray(ln_gamma, np.float32)
    bet = np.asarray(ln_beta, np.float32)
    wq_, wk_, wv_ = w_qkv[0:C], w_qkv[C:2 * C], w_qkv[2 * C:3 * C]

    f8 = ml_dtypes.float8_e4m3fn

    def prep(wmat, scale):
        wg = (scale * wmat * gam[None, :]).astype(f8)           # [o, c] gamma folded
        wT = wg.T.astype(np.float32)                            # lhsT layout [in, out]
        # fold ci tiles: dram [128, 2, C] with (p, ci, o) = wT[ci*128 + p, o]
        wf = np.stack([wT[0:128], wT[128:256]], axis=1).astype(f8)
        sw = wg.astype(np.float32).sum(1)                       # rowsum of device weights
        bias = scale * (wmat @ bet)
        return (np.ascontiguousarray(wf.reshape(128, 2 * C)),
                np.ascontiguousarray(np.stack([bias, -sw]).astype(bf)))

    wq8, wbq_h = prep(wq_, SQA)
    wk8, wbk_h = prep(wk_, SQA)
    # V extended with a zero-weight dummy channel per head whose rank-1 bias
    # is 1 against the ones row (becomes the softmax-denominator column).
    wvg = (wv_ * gam[None, :]).astype(f8)
    wv_ext = np.zeros((NH * 33, C), f8)
    wbv_h = np.zeros((2, NH * 33), np.float32)
    for h in range(NH):
        wv_ext[h * 33:h * 33 + 32] = wvg[h * 32:(h + 1) * 32]
        wbv_h[1, h * 33:h * 33 + 32] = -wvg[h * 32:(h + 1) * 32].astype(np.float32).sum(1)
        wbv_h[0, h * 33:h * 33 + 32] = (wv_ @ bet)[h * 32:(h + 1) * 32]
        wbv_h[0, h * 33 + 32] = 1.0
    wvT = np.ascontiguousarray(wv_ext.T)
    wbv_h = wbv_h.astype(bf)
    # out-proj tiles matching attx layout: tile t=(hg,pr): rows 0:32 = head
    # hg*4+2pr channels, rows 64:96 = head hg*4+2pr+1; other rows zero.
    wpT = np.asarray(w_proj, np.float32).T  # [in=attn ch, out]
    wpx = np.zeros((4 * 128, C), np.float32)
    for hg in range(2):
        for pr in range(2):
            t = hg * 2 + pr
            h0 = hg * 4 + 2 * pr
            wpx[t * 128 + 0:t * 128 + 32] = wpT[h0 * 32:(h0 + 1) * 32]
            wpx[t * 128 + 64:t * 128 + 96] = wpT[(h0 + 1) * 32:(h0 + 2) * 32]
    wpx = np.ascontiguousarray(wpx.astype(bf))
    bp = np.asarray(b_proj, np.float32).reshape(C, 1)

    xfull = x.reshape(B, C, N)
    in_maps = []
    for core in range(8):
        b, qc = core // 4, core % 4
        xr_ = np.roll(xfull[b], -qc * NQ, axis=1)
        in_maps.append({
            "xb": np.ascontiguousarray(xr_.astype(bf)),
            "xf": np.ascontiguousarray(xr_[:, :NQ]),
            "wq8": wq8, "wk8": wk8, "wvT": wvT, "wpx": wpx,
            "wbq": wbq_h, "wbk": wbk_h, "wbv": wbv_h, "bp": bp,
        })
    return in_maps


def kernel(x, ln_gamma, ln_beta, w_qkv, w_proj, b_proj):
    from concourse.bass_utils import run_bass_kernel_spmd

    if "nc" not in _CACHE:
        _CACHE["nc"] = _build()
    nc = _CACHE["nc"]

    in_maps = _prep_in_maps(x, ln_gamma, ln_beta, w_qkv, w_proj, b_proj)
    res = run_bass_kernel_spmd(nc, in_maps, core_ids=list(range(8)),
                               trace=_PROFILE)
    if _PROFILE:
        _CACHE["exec_time_ns"] = res.exec_time_ns
    out = np.empty((B, C, N), np.float32)
    for core in range(8):
        b, qc = core // 4, core % 4
        out[b][:, qc * NQ:(qc + 1) * NQ] = res.results[core]["out"]
    return out.reshape(B, C, H, W)


# revision 42
# speedup vs baseline: 1.0072x; 1.0072x over previous
"""Trainium2 Bass kernel for nn_Attention (B=2, C=256, H=W=64, 8 heads).

Sharding: 8 cores = 2 batches x 4 query-chunks (1024 queries each), no
collectives. Each core gets its batch's full x (bf16) with token columns
rolled so its own query chunk sits at columns 0:1024 (attention is
permutation-invariant over keys); it computes LN + projections + attention
for its queries and writes a [256, 1024] slice of the output.

v3 structure (DoubleRow fp8 S matmuls; ~390us -> ~2xx us sim):
- x is pre-scaled by rstd (x_rstd = x * rstd broadcast) so every projection
  PSUM holds the final value directly: W @ x_rstd - rowsum(W) x (mu*rstd)
  + (W@beta) x 1. Projection evacuations become plain dtype-converting
  copies (no per-token multiply), freeing DVE time.
- K and Q projections are emitted in a folded [64, 2, 512] PSUM layout
  (channel ch = p + 64*i) by splitting each projection into two 64-column
  stationaries. Evacuated to fp8 [128, 2, N] tiles (upper 64 partitions
  zero), which feed full-array *unmasked* DoubleRow fp8 S matmuls:
  256 cycles per [128 keys, 512 queries] head-tile — 2x the bf16 rate,
  while still counting as PE activity for the HAM clock gate (masked
  tile_position matmuls don't, and would drop the whole phase to 1.2 GHz).
- Wq and Wk each carry sqrt(128*log2e*attn_scale) so PSUM logits keep the
  Schraudolph convention: pair-0 exp via f32->i16 convert bit trick on
  VectorE, pair-1 true Exp on ScalarE, both [128,1024] per key-chunk.
- S pair-0 PSUM is double-buffered (pair-1 single) so the j-loop period is
  set by engine capacity (~1.3us: PE 4xDR-S + 4xAV, DVE exp, ACT exp), not
  the S->exp->S bank turnaround.
- AV keeps the 33rd dummy-V-channel trick (bias row pairs the ones row) so
  softmax denominators accumulate for free; AV for chunk j is emitted after
  the S matmuls of chunk j+1 (software pipelining) so the exp engines stay
  off the PE's serial path. Accumulators are evacuated RAW (valid partition
  ranges only) to per-(hg,pr) attnT tiles whose garbage rows are zeroed
  once; normalization (bit-trick reciprocal of the denominator rows, DMA
  hop to partition 0, gpsimd partition_broadcast, multiply) happens off the
  critical path. NOTE: hw partition_broadcast reads the tile's partition 0
  and ignores the AP base partition — the DMA hop is required (reading the
  row at partition 32/96 directly returns uninitialized memory on hw, sim
  does not model this). The output projection contracts the four attnT
  tiles with zero-padded reordered W_proj tiles at the very end.

v3 measured: 247027 ns CoreSim (baseline 390555 sim / 443752 ns HW);
rel err 1.65e-4 on hardware, stable across runs. K/Q projections contract
all 256 input channels in ONE DoubleRow matmul each (x*rstd is written as
a ci-folded [128, 2, N] fp8 tile, weights ship host-folded fp8), so a
projection group is one 107 ns DR matmul plus the rank-2 fixup.
"""

import numpy as np

B, C, H, W = 2, 256, 64, 64
N = H * W            # 4096 tokens
NH, HD = 8, 32       # heads, head_dim
NQ = N // 4          # queries per core
LN_EPS = 1e-5
LOG2E = 1.4426950408889634
LN2 = 0.6931471805599453
ATTN_SCALE = HD ** -0.5
A_SCALE = 128.0 * LOG2E * ATTN_SCALE   # total logit scale in PSUM
SQA = A_SCALE ** 0.5                   # folded into each of Wq, Wk
B16F = 16256.0 - 5.6                   # Schraudolph bias (calibrated)
KRSQ = 24375.25                        # bf16 bit-trick rsqrt bias (<=3.7% rel)
KRCP = 32498.75                        # bf16 bit-trick recip bias (<=5.3% rel)

_PROFILE = False
_DEBUG = False
_CACHE = {}


def _build():
    from concourse import bacc
    from concourse import mybir
    import concourse.tile as tile

    f32 = mybir.dt.float32
    bf16 = mybir.dt.bfloat16
    fp8 = mybir.dt.float8e4
    i16 = mybir.dt.int16
    ALU = mybir.AluOpType
    ACTF = mybir.ActivationFunctionType
    DR = mybir.MatmulPerfMode.DoubleRow

    nc = bacc.Bacc("TRN2", target_bir_lowering=False)
    xbd = nc.dram_tensor("xb", [C, N], bf16, kind="ExternalInput")
    xfd = nc.dram_tensor("xf", [C, NQ], f32, kind="ExternalInput")
    wqd = nc.dram_tensor("wq8", [128, 2 * C], fp8, kind="ExternalInput")  # gamma+SQA, ci-folded
    wkd = nc.dram_tensor("wk8", [128, 2 * C], fp8, kind="ExternalInput")
    wvd = nc.dram_tensor("wvT", [C, NH * 33], fp8, kind="ExternalInput")
    wpxd = nc.dram_tensor("wpx", [4 * 128, C], bf16, kind="ExternalInput")
    # rank-1 LN fixup pairs: row0 = W@beta (pairs ones), row1 = -rowsum(W')
    # (pairs mu*rstd) — one contract-2 fixup matmul per projection chunk
    wbqd = nc.dram_tensor("wbq", [2, C], bf16, kind="ExternalInput")
    wbkd = nc.dram_tensor("wbk", [2, C], bf16, kind="ExternalInput")
    wbvd = nc.dram_tensor("wbv", [2, NH * 33], bf16, kind="ExternalInput")
    bpd = nc.dram_tensor("bp", [C, 1], f32, kind="ExternalInput")
    od = nc.dram_tensor("out", [C, NQ], f32, kind="ExternalOutput")
    if _DEBUG:
        dbg = {
            "d_kT0": nc.dram_tensor("d_kT0", [128, 2 * N], fp8, kind="ExternalOutput"),
            "d_qp0": nc.dram_tensor("d_qp0", [128, 2 * NQ], fp8, kind="ExternalOutput"),
            "d_musrt2": nc.dram_tensor("d_musrt2", [2, N], bf16, kind="ExternalOutput"),
            "d_vsb": nc.dram_tensor("d_vsb", [128, 32 * NH * 33], bf16, kind="ExternalOutput"),
            "d_attx0": nc.dram_tensor("d_attx0", [128, NQ], bf16, kind="ExternalOutput"),
            "d_attx1": nc.dram_tensor("d_attx1", [128, NQ], bf16, kind="ExternalOutput"),
            "d_attx2": nc.dram_tensor("d_attx2", [128, NQ], bf16, kind="ExternalOutput"),
            "d_attx3": nc.dram_tensor("d_attx3", [128, NQ], bf16, kind="ExternalOutput"),
            "d_rsbf": nc.dram_tensor("d_rsbf", [1, N], bf16, kind="ExternalOutput"),
            "d_kT1": nc.dram_tensor("d_kT1", [128, 2 * N], fp8, kind="ExternalOutput"),
            "d_qp4": nc.dram_tensor("d_qp4", [128, 2 * NQ], fp8, kind="ExternalOutput"),
            "d_qp7": nc.dram_tensor("d_qp7", [128, 2 * NQ], fp8, kind="ExternalOutput"),
        }

    with tile.TileContext(nc) as tc:
        with tc.tile_pool(name="big", bufs=1) as big, \
             tc.tile_pool(name="sml", bufs=4) as sml:

            # ---- load inputs ----
            xb = [big.tile([128, N], bf16, tag=f"xb{c}", name=f"xb{c}") for c in range(2)]
            for q4 in range(4):
                qs = slice(q4 * 1024, (q4 + 1) * 1024)
                for c in range(2):
                    nc.sync.dma_start(out=xb[c][:, qs], in_=xbd[c * 128:(c + 1) * 128, qs])
            xf = [big.tile([128, NQ], f32, tag=f"xf{c}", name=f"xf{c}") for c in range(2)]
            for c in range(2):
                nc.sync.dma_start(out=xf[c][:, :], in_=xfd[c * 128:(c + 1) * 128, :])
            w_sb = {}
            for name, t in (("q", wqd), ("k", wkd)):
                s = big.tile([128, 2, C], fp8, tag=f"w{name}8", name=f"w{name}8")
                nc.sync.dma_start(out=s[:, :, :].rearrange("p a b -> p (a b)"), in_=t[:, :])
                w_sb[name] = s
            for ci in range(2):
                s = big.tile([128, NH * 33], fp8, tag=f"wv{ci}", name=f"wv{ci}")
                nc.sync.dma_start(out=s[:, :], in_=wvd[ci * 128:(ci + 1) * 128, :])
                w_sb["v", ci] = s
            wpx_sb = [big.tile([128, C], bf16, tag=f"wpx{t}", name=f"wpx{t}") for t in range(4)]
            for t in range(4):
                nc.sync.dma_start(out=wpx_sb[t][:, :], in_=wpxd[t * 128:(t + 1) * 128, :])
            wb_sb = {}
            for name, t, nout in (("q", wbqd, C), ("k", wbkd, C), ("v", wbvd, NH * 33)):
                s = big.tile([2, nout], bf16, tag=f"wb{name}", name=f"wb{name}")
                nc.sync.dma_start(out=s[:, :], in_=t[:, :])
                wb_sb[name] = s
            bp_sb = [big.tile([128, 1], f32, tag=f"bp{c}", name=f"bp{c}") for c in range(2)]
            for c in range(2):
                nc.sync.dma_start(out=bp_sb[c][:, :], in_=bpd[c * 128:(c + 1) * 128, :])

            onesC = big.tile([128, 1], bf16, tag="onesC", name="onesC")
            nc.vector.memset(onesC[:, :], 1.0 / C)
            ones_row = big.tile([1, 128], bf16, tag="onesr", name="onesr")
            nc.vector.memset(ones_row[:, :], 1.0)

            mu_row = big.tile([1, N], bf16, tag="murow", name="murow")
            rs_bf = big.tile([1, N], bf16, tag="rsbf", name="rsbf")
            murs_row = big.tile([1, N], bf16, tag="mursrow", name="mursrow")
            musrt2 = big.tile([2, N], bf16, tag="musrt2", name="musrt2")
            nc.vector.memset(musrt2[0:1, :], 1.0)
            rs_ball = big.tile([128, N], bf16, tag="rsball", name="rsball")
            xr2 = big.tile([128, 2, N], fp8, tag="xr2", name="xr2")
            xsq = [big.tile([128, N], bf16, tag=f"xsq{c}", name=f"xsq{c}") for c in range(2)]

            # folded fp8 K (ch = p + 64*i within hg block; upper 64 partitions 0)
            kT = [big.tile([128, 2, N], fp8, tag=f"kT{c}", name=f"kT{c}") for c in range(2)]
            for c in range(2):
                nc.gpsimd.memset(kT[c][:, :, :], 0.0)
            # folded fp8 padded Q per head (only 32 rows of one i-half nonzero)
            qp = [big.tile([128, 2, NQ], fp8, tag=f"qp{h}", name=f"qp{h}") for h in range(NH)]
            for h in range(NH):
                nc.gpsimd.memset(qp[h][:, :, :], 0.0)
            v_sb = big.tile([128, 32, NH, 33], bf16, tag="v", name="v")
            # attn output tiles, rows 0:33 & 64:97 valid (head pair + denoms)
            attx = [big.tile([128, NQ], bf16, tag=f"at{t}", name=f"at{t}") for t in range(4)]
            for t in range(4):
                nc.gpsimd.memset(attx[t][:, :], 0.0)
            rcpT = big.tile([128, NQ], bf16, tag="rcpT", name="rcpT")

            # ---- LN stats + projections ----
            with tc.tile_pool(name="lnsb", bufs=1) as lnsb, \
                 tc.tile_pool(name="lnp", bufs=2, space="PSUM") as lnp, \
                 tc.tile_pool(name="rsp", bufs=1, space="PSUM") as rsp, \
                 tc.tile_pool(name="mmk", bufs=3, space="PSUM") as mmk, \
                 tc.tile_pool(name="mm", bufs=2, space="PSUM") as mmp:
                # full-array warmup matmuls on early-arriving weight tiles:
                # releases the HAM clock-gate to 2.4 GHz during the x DMA wait
                for w in range(20):
                    wps = mmp.tile([128, NH * 33], f32, tag="vproj", name="warm")
                    nc.tensor.matmul(wps[:, :], wpx_sb[w % 4][:, 0:128],
                                     w_sb["v", w % 2][:, :], start=True, stop=True)
                for hh in range(2):
                    hl = slice(hh * 2048, (hh + 1) * 2048)
                    for c in range(2):
                        nc.vector.tensor_tensor(xsq[c][:, hl], xb[c][:, hl],
                                                xb[c][:, hl], ALU.mult)
                # stats matmuls for all chunks; evacuate mu/sumsq rows to SBUF
                ssum = lnsb.tile([1, N], bf16, tag="ssum", name="ssum")
                for f in range(8):
                    fl = slice(f * 512, (f + 1) * 512)
                    mps = lnp.tile([1, 512], f32, tag="st", name="mps")
                    nc.tensor.matmul(mps[:, :], onesC[:, :], xb[0][:, fl], start=True, stop=False)
                    nc.tensor.matmul(mps[:, :], onesC[:, :], xb[1][:, fl], start=False, stop=True)
                    sps = lnp.tile([1, 512], f32, tag="st", name="sps")
                    nc.tensor.matmul(sps[:, :], onesC[:, :], xsq[0][:, fl], start=True, stop=False)
                    nc.tensor.matmul(sps[:, :], onesC[:, :], xsq[1][:, fl], start=False, stop=True)
                    nc.scalar.copy(mu_row[0:1, fl], mps[:, :])
                    nc.scalar.copy(ssum[0:1, fl], sps[:, :])
                # one-shot whole-row LN math, in halves so the first half's
                # projections can start while the second half still chains
                mu2 = lnsb.tile([1, N], bf16, tag="mu2", name="mu2")
                vare = lnsb.tile([1, N], bf16, tag="vare", name="vare")
                for hh in range(4):
                    hl = slice(hh * 1024, (hh + 1) * 1024)
                    nc.vector.tensor_tensor(mu2[0:1, hl], mu_row[0:1, hl],
                                            mu_row[0:1, hl], ALU.mult)
                    nc.vector.tensor_tensor(vare[0:1, hl], ssum[0:1, hl],
                                            mu2[0:1, hl], ALU.subtract)
                    # rstd via bf16 exponent bit trick on VectorE
                    nc.vector.tensor_scalar(rs_bf[0:1, hl].bitcast(i16),
                                            vare[0:1, hl].bitcast(i16),
                                            -0.5, KRSQ, ALU.mult, ALU.add)
                    # mu*rstd -> musrt2 row1 (engines cannot write partition 1,
                    # but an SBUF->SBUF DMA can)
                    nc.vector.tensor_tensor(murs_row[0:1, hl], mu_row[0:1, hl],
                                            rs_bf[0:1, hl], ALU.mult)
                    nc.sync.dma_start(out=musrt2[1:2, hl], in_=murs_row[0:1, hl])
                # rstd broadcast + x*rstd for every chunk, ahead of projections
                for f in range(8):
                    fl = slice(f * 512, (f + 1) * 512)
                    rsb_ps = rsp.tile([128, 512], f32, tag="rsb", name="rsb")
                    nc.tensor.matmul(rsb_ps[:, :], ones_row[:, :], rs_bf[0:1, fl],
                                     start=True, stop=True)
                    nc.scalar.copy(rs_ball[:, fl], rsb_ps[:, :])
                    for c in range(2):
                        nc.vector.tensor_tensor(xr2[:, c, fl], xb[c][:, fl],
                                                rs_ball[:, fl], ALU.mult)
                for f in range(8):
                    fl = slice(f * 512, (f + 1) * 512)
                    # K projection, folded [64, 512] per (hg, i) (ch = p + 64*i)
                    for hg in range(2):
                        for i in range(2):
                            ps = mmk.tile([64, 512], f32, tag="proj", name="kproj")
                            cs = slice(hg * 128 + i * 64, hg * 128 + (i + 1) * 64)
                            nc.tensor.matmul(ps[:, :], w_sb["k"][:, :, cs],
                                             xr2[:, :, fl], perf_mode=DR,
                                             start=True, stop=False)
                            nc.tensor.matmul(ps[:, :], wb_sb["k"][:, cs],
                                             musrt2[:, fl], start=False, stop=True)
                            nc.scalar.copy(kT[hg][0:64, i, fl], ps[:, :])

                    # Q projection (first two chunks = this core's queries)
                    if f < 2:
                        for hg in range(2):
                            for i in range(2):
                                ps = mmk.tile([64, 512], f32, tag="proj", name="qproj")
                                cs = slice(hg * 128 + i * 64, hg * 128 + (i + 1) * 64)
                                nc.tensor.matmul(ps[:, :], w_sb["q"][:, :, cs],
                                                 xr2[:, :, fl], perf_mode=DR,
                                                 start=True, stop=False)
                                nc.tensor.matmul(ps[:, :], wb_sb["q"][:, cs],
                                                 musrt2[:, fl], start=False, stop=True)
                                # heads 2i, 2i+1 of hg live in rows 0:32, 32:64
                                for mh in range(2):
                                    m = i * 2 + mh
                                    rr = slice(32 * mh, 32 * mh + 32)
                                    nc.vector.tensor_copy(qp[hg * 4 + m][rr, i, fl],
                                                          ps[rr, :])

                # V projection per 128-token chunk (tokens in partitions). The
                # 33rd "dummy" channel per head has zero weights and rank-1
                # bias = 1 (pairs the ones row), so it accumulates the softmax
                # denominator during AV.
                for j in range(32):
                    jl = slice(j * 128, (j + 1) * 128)
                    ps = mmp.tile([128, NH * 33], f32, tag="vproj", name="vproj")
                    for ci in range(2):
                        nc.tensor.matmul(ps[:, :], xr2[:, ci, jl], w_sb["v", ci][:, :],
                                         start=(ci == 0), stop=False)
                    nc.tensor.matmul(ps[:, :], musrt2[:, jl],
                                     wb_sb["v"][:, :], start=False, stop=True)
                    nc.scalar.copy(v_sb[:, j, :, :],
                                   ps[:, :].rearrange("p (h e) -> p h e", h=NH))

            # ---- attention ----
            with tc.tile_pool(name="ssp", bufs=3, space="PSUM") as ssp, \
                 tc.tile_pool(name="avp", bufs=1, space="PSUM") as avp, \
                 tc.tile_pool(name="pp", bufs=3) as ppool, \
                 tc.tile_pool(name="nrm", bufs=4) as nrm:
                for f in range(2):
                    fl = slice(f * 512, (f + 1) * 512)
                    for hg in range(2):
                        av = [avp.tile([128, 512], f32, tag=f"av{pr}", name=f"av{pr}")
                              for pr in range(2)]

                        def emit_av(j, pt):
                            for pr in range(2):
                                for t2 in range(2):
                                    h = pr * 2 + t2
                                    nc.tensor.matmul(
                                        av[pr][t2 * 64:t2 * 64 + 33, :],
                                        v_sb[:, j, hg * 4 + h, :],
                                        pt[pr][:, t2 * 512:(t2 + 1) * 512],
                                        start=(j == 0), stop=(j == 31),
                                        skip_group_check=True,
                                        tile_position=(0, t2 * 64))

                        # software-pipelined: AV for chunk j is emitted after
                        # the S matmuls of chunk j+1 so the exp engines are
                        # never on the PE's serial path
                        pending = None
                        for j in range(32):
                            jl = slice(j * 128, (j + 1) * 128)
                            ss = [ssp.tile([128, 1024], f32, tag="ss", name="s0"),
                                  ssp.tile([128, 1024], f32, tag="ss", name="s1")]
                            pt = [ppool.tile([128, 1024], bf16, tag=f"p{i}", name=f"p{i}")
                                  for i in range(2)]
                            for pr in range(2):
                                for t2 in range(2):
                                    h = hg * 4 + pr * 2 + t2
                                    nc.tensor.matmul(ss[pr][:, t2 * 512:(t2 + 1) * 512],
                                                     kT[hg][:, :, jl], qp[h][:, :, fl],
                                                     perf_mode=DR, start=True, stop=True)
                            # pair 0: Schraudolph on VectorE; pair 1: ScalarE Exp
                            nc.vector.tensor_scalar(pt[0][:, :].bitcast(i16), ss[0][:, :],
                                                    B16F, None, ALU.add)
                            nc.scalar.activation(pt[1][:, :], ss[1][:, :],
                                                 ACTF.Exp, scale=LN2 / 128.0)
                            if pending is not None:
                                emit_av(*pending)
                            pending = (j, pt)
                        emit_av(*pending)
                        # raw evacuation (valid rows only; garbage rows stay 0)
                        for pr in range(2):
                            t = hg * 2 + pr
                            nc.scalar.copy(attx[t][0:33, fl], av[pr][0:33, :])
                            nc.scalar.copy(attx[t][64:97, fl], av[pr][64:97, :])
                            # denominator reciprocals (bf16 bit trick), then
                            # 0-stride-DMA broadcast + multiply off critical path
                            for t2 in range(2):
                                r = 32 + 64 * t2
                                nc.vector.tensor_scalar(rcpT[r:r + 1, fl].bitcast(i16),
                                                        attx[t][r:r + 1, fl].bitcast(i16),
                                                        -1.0, KRCP, ALU.mult, ALU.add)
                                # hop the reciprocal row to partition 0 via DMA
                                # (hw partition_broadcast reads the tile's
                                # partition 0, ignoring the AP base partition)
                                rcp0 = nrm.tile([1, 512], bf16, tag="rcp0", name="rcp0")
                                nc.sync.dma_start(out=rcp0[:, :], in_=rcpT[r:r + 1, fl])
                                bcs = nrm.tile([128, 512], bf16, tag="bcs", name="bcs")
                                nc.gpsimd.partition_broadcast(bcs[:, :], rcp0[:, :])
                                nc.vector.tensor_tensor(attx[t][64 * t2:64 * t2 + 32, fl],
                                                        attx[t][64 * t2:64 * t2 + 32, fl],
                                                        bcs[64 * t2:64 * t2 + 32, :], ALU.mult)

            if _DEBUG:
                nc.sync.dma_start(out=dbg["d_kT0"][:, :],
                                  in_=kT[0][:, :, :].rearrange("p a b -> p (a b)"))
                nc.sync.dma_start(out=dbg["d_qp0"][:, :],
                                  in_=qp[0][:, :, :].rearrange("p a b -> p (a b)"))
                nc.sync.dma_start(out=dbg["d_musrt2"][:, :], in_=musrt2[:, :])
                nc.sync.dma_start(out=dbg["d_vsb"][:, :],
                                  in_=v_sb[:, :, :, :].rearrange("p a b c -> p (a b c)"))
                for _t in range(4):
                    nc.sync.dma_start(out=dbg[f"d_attx{_t}"][:, :], in_=attx[_t][:, :])
                nc.sync.dma_start(out=dbg["d_rsbf"][:, :], in_=rs_bf[:, :])
                nc.sync.dma_start(out=dbg["d_kT1"][:, :],
                                  in_=kT[1][:, :, :].rearrange("p a b -> p (a b)"))
                nc.sync.dma_start(out=dbg["d_qp4"][:, :],
                                  in_=qp[4][:, :, :].rearrange("p a b -> p (a b)"))
                nc.sync.dma_start(out=dbg["d_qp7"][:, :],
                                  in_=qp[7][:, :, :].rearrange("p a b -> p (a b)"))

            # ---- output projection + bias + residual ----
            with tc.tile_pool(name="mm2", bufs=2, space="PSUM") as mm2, \
                 tc.tile_pool(name="ot", bufs=4) as otp:
                for mo in range(2):
                    ms = slice(mo * 128, (mo + 1) * 128)
                    for fh in range(2):
                        fl = slice(fh * 512, (fh + 1) * 512)
                        ps = mm2.tile([128, 512], f32, tag="o", name="o")
                        for t in range(4):
                            nc.tensor.matmul(ps[:, :], wpx_sb[t][:, ms],
                                             attx[t][:, fl], start=(t == 0), stop=(t == 3))
                        ot = otp.tile([128, 512], f32, tag="ot", name="ot")
                        nc.vector.scalar_tensor_tensor(ot[:, :], ps[:, :], bp_sb[mo][:, :],
                                                       xf[mo][:, fl], ALU.add, ALU.add)
                        nc.sync.dma_start(out=od[ms, fl], in_=ot[:, :])

    nc.finalize()
    return nc


def _prep_in_maps(x, ln_gamma, ln_beta, w_qkv, w_proj, b_proj):
    import ml_dtypes

    bf = ml_dtypes.bfloat16
    x = np.asarray(x, np.float32)
    w_qkv = np.asarray(w_qkv, np.float32)
    gam = np.asarray(ln_gamma, np.float32)
    bet = np.asarray(ln_beta, np.float32)
    wq_, wk_, wv_ = w_qkv[0:C], w_qkv[C:2 * C], w_qkv[2 * C:3 * C]

    f8 = ml_dtypes.float8_e4m3fn

    def prep(wmat, scale):
        wg = (scale * wmat * gam[None, :]).astype(f8)           # [o, c] gamma folded
        wT = wg.T.astype(np.float32)                            # lhsT layout [in, out]
        # fold ci tiles: dram [128, 2, C] with (p, ci, o) = wT[ci*128 + p, o]
        wf = np.stack([wT[0:128], wT[128:256]], axis=1).astype(f8)
        sw = wg.astype(np.float32).sum(1)                       # rowsum of device weights
        bias = scale * (wmat @ bet)
        return (np.ascontiguousarray(wf.reshape(128, 2 * C)),
                np.ascontiguousarray(np.stack([bias, -sw]).astype(bf)))

    wq8, wbq_h = prep(wq_, SQA)
    wk8, wbk_h = prep(wk_, SQA)
    # V extended with a zero-weight dummy channel per head whose rank-1 bias
    # is 1 against the ones row (becomes the softmax-denominator column).
    wvg = (wv_ * gam[None, :]).astype(f8)
    wv_ext = np.zeros((NH * 33, C), f8)
    wbv_h = np.zeros((2, NH * 33), np.float32)
    for h in range(NH):
        wv_ext[h * 33:h * 33 + 32] = wvg[h * 32:(h + 1) * 32]
        wbv_h[1, h * 33:h * 33 + 32] = -wvg[h * 32:(h + 1) * 32].astype(np.float32).sum(1)
        wbv_h[0, h * 33:h * 33 + 32] = (wv_ @ bet)[h * 32:(h + 1) * 32]
        wbv_h[0, h * 33 + 32] = 1.0
    wvT = np.ascontiguousarray(wv_ext.T)
    wbv_h = wbv_h.astype(bf)
    # out-proj tiles matching attx layout: tile t=(hg,pr): rows 0:32 = head
    # hg*4+2pr channels, rows 64:96 = head hg*4+2pr+1; other rows zero.
    wpT = np.asarray(w_proj, np.float32).T  # [in=attn ch, out]
    wpx = np.zeros((4 * 128, C), np.float32)
    for hg in range(2):
        for pr in range(2):
            t = hg * 2 + pr
            h0 = hg * 4 + 2 * pr
            wpx[t * 128 + 0:t * 128 + 32] = wpT[h0 * 32:(h0 + 1) * 32]
            wpx[t * 128 + 64:t * 128 + 96] = wpT[(h0 + 1) * 32:(h0 + 2) * 32]
    wpx = np.ascontiguousarray(wpx.astype(bf))
    bp = np.asarray(b_proj, np.float32).reshape(C, 1)

    xfull = x.reshape(B, C, N)
    in_maps = []
    for core in range(8):
        b, qc = core // 4, core % 4
        xr_ = np.roll(xfull[b], -qc * NQ, axis=1)
        in_maps.append({
            "xb": np.ascontiguousarray(xr_.astype(bf)),
            "xf": np.ascontiguousarray(xr_[:, :NQ]),
            "wq8": wq8, "wk8": wk8, "wvT": wvT, "wpx": wpx,
            "wbq": wbq_h, "wbk": wbk_h, "wbv": wbv_h, "bp": bp,
        })
    return in_maps


def kernel(x, ln_gamma, ln_beta, w_qkv, w_proj, b_proj):
    from concourse.bass_utils import run_bass_kernel_spmd

    if "nc" not in _CACHE:
        _CACHE["nc"] = _build()
    nc = _CACHE["nc"]

    in_maps = _prep_in_maps(x, ln_gamma, ln_beta, w_qkv, w_proj, b_proj)
    res = run_bass_kernel_spmd(nc, in_maps, core_ids=list(range(8)),
                               trace=_PROFILE)
    if _PROFILE:
        _CACHE["exec_time_ns"] = res.exec_time_ns
    out = np.empty((B, C, N), np.float32)
    for core in range(8):
        b, qc = core // 4, core % 4
        out[b][:, qc * NQ:(qc + 1) * NQ] = res.results[core]["out"]
    return out.reshape(B, C, H, W)
